# revision 1
# baseline (speedup 1.0000x reference)
"""v1: fp8 DoubleRow residual kernel — 5-slot tiles, per-slot DMAs,
56 single-bank PSUM passes, 56 per-pass drains, outs per (b,m,half)."""

import numpy as np
import ml_dtypes

import concourse.bass as bass
import concourse.tile as tile
from concourse import bacc, mybir
from concourse.bass_utils import run_bass_kernel_spmd

FP8 = ml_dtypes.float8_e4m3

N_CORES = 8
C = 512
MT = C // 128
FCH = 448
ALPHA = 128.0
X_TARGET = 224.0
BETA = 0.25

_CACHE = {}


def _build_program(n_batch_per_core: int, hw: int, c_drain: float):
    nfc = hw // FCH
    assert nfc * FCH == hw and hw % 16 == 0
    nfa = (nfc + 1) // 2
    ca = nfa * FCH
    nc = bacc.Bacc(
        "TRN2", target_bir_lowering=False, debug=False, enable_asserts=False
    )
    x_d = nc.dram_tensor(
        "x", [n_batch_per_core, C, hw], mybir.dt.float8e4, kind="ExternalInput"
    ).ap()
    w_d = nc.dram_tensor(
        "w", [128, MT, 2, 128], mybir.dt.float8e4, kind="ExternalInput"
    ).ap()
    d_d = nc.dram_tensor(
        "d", [n_batch_per_core, C, hw], mybir.dt.int8, kind="ExternalOutput"
    ).ap()

    with tile.TileContext(nc) as tc:
        with (
            tc.tile_pool(name="w", bufs=1) as w_pool,
            tc.tile_pool(name="x", bufs=1) as x_pool,
            tc.tile_pool(name="ps", bufs=3, space="PSUM") as ps_pool,
            tc.tile_pool(name="pst", bufs=2, space="PSUM") as pst_pool,
            tc.tile_pool(name="out", bufs=3) as out_pool,
        ):
            wsb = w_pool.tile([128, MT, 2, 128], mybir.dt.float8e4, tag="w")
            nc.sync.dma_start(wsb[:], w_d)

            xs = []
            for b in range(n_batch_per_core):
                xs.append(
                    x_pool.tile(
                        [128, 4, hw], mybir.dt.float8e4, tag=f"x{b}", name=f"x{b}"
                    )
                )
            for b in range(n_batch_per_core):
                for s in range(4):
                    nc.sync.dma_start(
                        xs[b][:, s, :], x_d[b, 128 * s : 128 * s + 128, :]
                    )

            # chunk grid per (b, m): 6 full 512-col chunks + one 64-col
            # tail; chunk pairs share a contiguous 2-bank PSUM tile drained
            # by a single flat op; one whole-row output DMA per (b, m).
            FB = 512
            nfull = hw // FB  # 6
            rem = hw - nfull * FB  # 64
            nd = 0
            for b in range(n_batch_per_core):
                for m in range(MT):
                    o = out_pool.tile(
                        [128, hw], mybir.dt.int8, tag="out", name=f"o{b}_{m}"
                    )

                    def _rhs(c0, c1, m=m, b=b):
                        return (
                            xs[b][:, m : m + 2, c0:c1]
                            if m < MT - 1
                            else xs[b][:, 3::-3, c0:c1]
                        )

                    for pr in range(nfull // 2):
                        ps = ps_pool.tile(
                            [128, 2, FB], mybir.dt.float32, tag="ps",
                            name=f"ps{b}_{m}_{pr}",
                        )
                        for i in range(2):
                            c0 = FB * (2 * pr + i)
                            nc.tensor.matmul(
                                ps[:, i, :],
                                wsb[:, m, :, :],
                                _rhs(c0, c0 + FB),
                                start=True,
                                stop=True,
                                perf_mode=mybir.MatmulPerfMode.DoubleRow,
                            )
                        dst = o[:, 2 * FB * pr : 2 * FB * (pr + 1)]
                        src = ps[:].rearrange("p a b -> p (a b)")
                        if nd % 2 == 0:
                            nc.vector.tensor_scalar_mul(dst, src, c_drain)
                        else:
                            nc.scalar.mul(dst, src, c_drain)
                        nd += 1
                    nc.sync.dma_start(
                        d_d[b, 128 * m : 128 * (m + 1), : nfull * FB],
                        o[:, : nfull * FB],
                    )
                    if rem and (b, m) == (n_batch_per_core - 1, 0):
                        # all x tiles are resident: run every (b, m) 64-col
                        # tail into one PSUM bank, drained once, shipped by
                        # one strided DMA; overlaps the remaining m blocks.
                        pst = pst_pool.tile(
                            [128, n_batch_per_core * MT, rem],
                            mybir.dt.float32, tag="pst", name="pst",
                        )
                        for bb in range(n_batch_per_core):
                            for mm in range(MT):
                                rhs = (
                                    xs[bb][:, mm : mm + 2, nfull * FB : hw]
                                    if mm < MT - 1
                                    else xs[bb][:, 3::-3, nfull * FB : hw]
                                )
                                nc.tensor.matmul(
                                    pst[:, bb * MT + mm, :],
                                    wsb[:, mm, :, :],
                                    rhs,
                                    start=True,
                                    stop=True,
                                    perf_mode=mybir.MatmulPerfMode.DoubleRow,
                                )
                        ot = out_pool.tile(
                            [128, n_batch_per_core * MT, rem],
                            mybir.dt.int8, tag="otail", name="otail",
                        )
                        nc.vector.tensor_scalar_mul(
                            ot[:],
                            pst[:].rearrange("p a b -> p (a b)").rearrange(
                                "p (a b) -> p a b", a=n_batch_per_core * MT
                            ),
                            c_drain,
                        )
                        nc.sync.dma_start(
                            d_d[:, :, nfull * FB : hw].rearrange(
                                "b (s p) c -> p (b s) c", s=MT
                            ),
                            ot[:],
                        )

    try:
        main_blk = nc.main_func.blocks[0]
        sp = mybir.EngineType.SP
        moved = None
        for blk in nc.main_func.blocks[1:]:
            cand = [
                i
                for i in blk.instructions
                if i.engine == sp
                and isinstance(i, mybir.InstDMACopy)
                and not (i.sync_info and i.sync_info.on_wait)
            ]
            if cand:
                moved = cand[:16]
                for i in moved:
                    blk.instructions.remove(i)
                break
        if moved:
            pos = next(
                idx
                for idx, i in enumerate(main_blk.instructions)
                if i.engine == sp and isinstance(i, mybir.InstDrain)
            )
            main_blk.instructions[pos:pos] = moved
    except Exception:
        pass

    for blk in nc.main_func.blocks:
        blk.instructions[:] = [
            inst
            for inst in blk.instructions
            if not (
                isinstance(inst, mybir.InstMemset)
                and inst.outs
                and "const-" in str(inst.outs[0])
            )
        ]
    nc.compile()
    return nc


def _residual_matrix(inhibition_filter: np.ndarray, c: int) -> np.ndarray:
    scope = inhibition_filter.shape[0]
    k = np.zeros(c, np.float64)
    k[:scope] = inhibition_filter.astype(np.float64)
    k = np.roll(k, -(scope // 2))
    delta = np.zeros(c, np.float64)
    delta[0] = 1.0
    g = np.fft.ifft(1.0 / np.fft.fft(delta - k)).real
    idx = (np.arange(c)[:, None] - np.arange(c)[None, :]) % c
    return g[idx] - np.eye(c)


def _pack_weights(B: np.ndarray) -> np.ndarray:
    W = np.zeros((128, MT, 2, 128), np.float64)
    r = np.arange(128)
    kk = np.arange(128)
    for m in range(MT):
        cout = 128 * m + r
        for j in range(2):
            cin = (128 * (m + j) - 64 + kk) % C
            W[:, m, j, :] = ALPHA * B[np.ix_(cout, cin)].T
    return W.astype(FP8)


def _reset_device():
    try:
        import ctypes

        import jax

        jax.devices()
        lib = ctypes.CDLL("/opt/axon/libaxon_pjrt.so")
        if hasattr(lib, "axon_reset"):
            lib.axon_reset.restype = ctypes.c_int64
            lib.axon_reset()
    except Exception:
        pass


def kernel(activations: np.ndarray, inhibition_filter: np.ndarray) -> np.ndarray:
    return _run(activations, inhibition_filter, trace=False)[0]


def _run(activations, inhibition_filter, trace=False):
    activations = np.ascontiguousarray(activations, dtype=np.float32)
    n, c, h, w_ = activations.shape
    assert c == C and n % N_CORES == 0
    hw = h * w_
    npc = n // N_CORES

    x = activations.reshape(n, c, hw)
    maxx = float(np.abs(x).max())
    s_x = maxx / X_TARGET
    s_d = BETA * maxx / 127.0
    c_drain = s_x / (ALPHA * s_d)

    B = _residual_matrix(np.asarray(inhibition_filter, np.float32), c)
    wq = _pack_weights(B)

    xr = np.concatenate([x[:, -64:, :], x[:, :-64, :]], axis=1)
    xq = (xr * (1.0 / s_x)).astype(FP8)
    xq = np.ascontiguousarray(xq.reshape(N_CORES, npc, c, hw))

    key = (npc, hw, round(c_drain, 12))
    if key not in _CACHE:
        _CACHE[key] = _build_program(npc, hw, c_drain)
    nc = _CACHE[key]

    in_maps = [{"x": xq[i], "w": wq} for i in range(N_CORES)]
    try:
        res = run_bass_kernel_spmd(nc, in_maps, list(range(N_CORES)), trace=trace)
    except Exception:
        _reset_device()
        res = run_bass_kernel_spmd(nc, in_maps, list(range(N_CORES)), trace=trace)
    d = np.stack([res.results[i]["d"] for i in range(N_CORES)])
    d = d.reshape(n, c, hw)
    y = x + d.astype(np.float32) * np.float32(s_d)
    return y.reshape(n, c, h, w_).astype(np.float32, copy=False), res

